# revision 42
# baseline (speedup 1.0000x reference)
"""GRU sequence model kernel for Trainium2 (8 NeuronCores, data-parallel).

Computes, per core (batch shard of 512):
    gi = x @ w_ih.T + b_ih            # done per-timestep, fused in loop
    h_{t+1} = GRU-cell(gi_t, h_t)     # 50 steps, hidden 512
    out = h_T @ w_out.T + b_out

Layout strategy: hidden state and all gate tensors live transposed on chip
([gate/hidden dim on partitions, batch on free dim]) so the recurrent matmul,
activations and elementwise updates need no per-step data movement. x ships
to the device in its natural [batch, time, input] layout as bf16 (halving the
host->device transfer, which dominates wall time) and is transposed on the
fly by the DMA XBAR (dma transpose, 2-byte dtypes) straight into the [input,
batch] SBUF layout the PE needs. Input-side matmuls run bf16*bf16 (PSUM
accumulates fp32); the recurrent path stays float32r end to end.
"""

import sys
from contextlib import ExitStack

import ml_dtypes
import numpy as np

sys.path.insert(0, "/opt/trn_rl_repo")

import jax  # noqa: E402

try:
    jax.config.update("jax_compilation_cache_dir", "/tmp/jax_comp_cache")
    jax.config.update("jax_persistent_cache_min_compile_time_secs", 0.0)
    jax.config.update("jax_persistent_cache_min_entry_size_bytes", 0)
except Exception:
    pass

import concourse.bass as bass  # noqa: E402
import concourse.tile as tile  # noqa: E402
from concourse import bacc, mybir  # noqa: E402
from concourse.bass_utils import run_bass_kernel_spmd  # noqa: E402

P = 128
T_STEPS = 50
T_I8 = 45  # timesteps 0..T_I8-1 ship as int8 codes; the rest as bf16.
# GRU gating contracts early-step quantization error to nothing: int8 for
# the first 45 steps + bf16 for the last 5 measures 3.6e-3 max rel err
# (all-bf16 is 3.7e-3) at 0.55x the bytes.
B_LOCAL = 512  # batch per core
I_DIM = 256  # input dim  (2 k-chunks)
H_DIM = 512  # hidden dim (4 k-chunks)
G_DIM = 1536  # 3*H gates  (12 chunks)
O_DIM = 256  # output dim
N_CORES = 8
N_HALVES = 2  # batch pipeline stages per step (1 = full batch per group)
BH = B_LOCAL // N_HALVES

F32 = mybir.dt.float32
F32R = mybir.dt.float32r
BF16 = mybir.dt.bfloat16
INT8 = mybir.dt.int8
AF = mybir.ActivationFunctionType
ALU = mybir.AluOpType


def _r(ap):
    """Matmul operand tiles are declared float32r; passthrough."""
    return ap


W_F32 = H_DIM * G_DIM + H_DIM * O_DIM  # w_hh_t + w_out_t, flat f32 blob
W_BF = I_DIM * G_DIM  # w_ih_t, flat bf16 blob

# All per-core inputs ride in ONE uint8 blob: the axon tunnel charges
# ~50-70ms latency PER ARRAY (76MB moves in 0.6s as one array vs 1.05s as
# six), so consolidating inputs buys ~0.4s. Regions are 4-byte aligned;
# the device carves them out with AP.bitcast.
_WS = W_F32 // N_CORES  # f32 elems
_WBS = W_BF // N_CORES  # bf16 elems
N_BIAS = P * 18  # f32 elems
XBF_N = B_LOCAL * (T_STEPS - T_I8) * I_DIM  # bf16 elems
X8_N = B_LOCAL * T_I8 * I_DIM  # int8 elems
OFF_BIAS_B = _WS * 4
OFF_WBSL_B = OFF_BIAS_B + N_BIAS * 4
OFF_XBF_B = OFF_WBSL_B + _WBS * 2
OFF_X8_B = OFF_XBF_B + XBF_N * 2
BLOB_BYTES = OFF_X8_B + X8_N


def _emit(ctx: ExitStack, tc: tile.TileContext, x8_d, xbf_d, wsl_d, wbsl_d, wslb_d, wbslb_d, wg_d, wgb_d, bias_d, out_d, n_steps):
    nc = tc.nc
    KI = I_DIM // P  # 2
    KH = H_DIM // P  # 4
    NB = B_LOCAL // P  # 4 batch chunks

    consts = ctx.enter_context(tc.tile_pool(name="consts", bufs=1))
    xtp8 = ctx.enter_context(tc.tile_pool(name="xtp8", bufs=2))
    xtpb = ctx.enter_context(tc.tile_pool(name="xtpb", bufs=2))
    xtq = ctx.enter_context(tc.tile_pool(name="xtq", bufs=3))
    gates = ctx.enter_context(tc.tile_pool(name="gates", bufs=6))
    ps_r = ctx.enter_context(tc.tile_pool(name="ps_r", bufs=2, space="PSUM"))
    ps_z = ctx.enter_context(tc.tile_pool(name="ps_z", bufs=2, space="PSUM"))
    ps_in = ctx.enter_context(tc.tile_pool(name="ps_in", bufs=2, space="PSUM"))
    ps_hn = ctx.enter_context(tc.tile_pool(name="ps_hn", bufs=2, space="PSUM"))

    # --- weights arrive as per-core 1/8 slices; AllGather them on-device.
    # Collectives may not read IO tensors, so bounce through Internal DRAM.
    nc.sync.dma_start(wslb_d, wsl_d)
    nc.sync.dma_start(wbslb_d, wbsl_d)
    nc.gpsimd.collective_compute(
        "AllGather", ALU.bypass, replica_groups=[list(range(N_CORES))],
        ins=[wslb_d.opt()], outs=[wg_d.opt()],
    )
    nc.gpsimd.collective_compute(
        "AllGather", ALU.bypass, replica_groups=[list(range(N_CORES))],
        ins=[wbslb_d.opt()], outs=[wgb_d.opt()],
    )

    # --- persistent SBUF tensors ---
    w_ih = consts.tile([P, KI, G_DIM], BF16, tag="w_ih")
    nc.sync.dma_start(w_ih[:], wgb_d.rearrange("(ko p g) -> p ko g", p=P, ko=KI, g=G_DIM))
    w_hh = consts.tile([P, KH, G_DIM], F32R, tag="w_hh")
    nc.sync.dma_start(
        w_hh[:], wg_d[0:H_DIM * G_DIM].rearrange("(ko p g) -> p ko g", p=P, ko=KH, g=G_DIM)
    )
    w_out = consts.tile([P, KH, O_DIM], F32R, tag="w_out")
    nc.sync.dma_start(
        w_out[:],
        wg_d[H_DIM * G_DIM:H_DIM * G_DIM + H_DIM * O_DIM].rearrange(
            "(ko p g) -> p ko g", p=P, ko=KH, g=O_DIM
        ),
    )
    biases = consts.tile([P, 18], F32, tag="biases")
    nc.sync.dma_start(biases[:], bias_d)

    # double-buffered hidden state, transposed layout [h-dim, batch].
    # One tile per 128-row chunk so matmul readers only depend on the chunk
    # they actually read. Double-buffering is essential: within a step,
    # later chunks' recurrent matmuls still read the OLD h of chunk 0.
    # The For_i body covers two steps so the buffer parity stays static.
    hbuf = [
        [
            [
                consts.tile([P, BH], F32R, tag=f"hbuf{i}_{a}_{c}", name=f"hbuf{i}_{a}_{c}")
                for c in range(KH)
            ]
            for a in range(N_HALVES)
        ]
        for i in range(2)
    ]

    def step(t, lbl, first, h_rd, h_wr, i8):
        """Emit one GRU step. t is a python int or a For_i index var.

        i8 steps index x8_d (int8 codes of x*s, dequantized on-chip by a
        DVE copy-cast, then transposed SBUF->SBUF by the DMA XBAR); bf16
        steps index xbf_d with t relative to T_I8 and transpose straight
        from DRAM. Both feed the same scaled-weight gi matmuls.
        """
        xT = xtq.tile([P, KI, B_LOCAL], BF16, tag="xT")
        if i8:
            xn8 = xtp8.tile([P, NB, I_DIM], INT8, tag="xn8")
            # natural [b, i] rows; separate queue from the XBAR transposes
            # so copy-mode/xbar-mode transitions don't serialize the stream
            nc.gpsimd.dma_start(
                xn8[:], x8_d[:, t, :].rearrange("(nb p) i -> p nb i", p=P)
            )
            xnb = xtpb.tile([P, NB, I_DIM], BF16, tag="xnb")
            nc.vector.tensor_copy(xnb[:], xn8[:])
            for ic in range(KI):
                for nb in range(NB):
                    nc.sync.dma_start(
                        out=xT[:, ic, nb * P:(nb + 1) * P],
                        in_=xnb[:, nb, ic * P:(ic + 1) * P],
                        transpose=True,
                    )
        else:
            x_t = xbf_d[:, t, :].rearrange("b (ic p) -> b ic p", p=P)
            for ic in range(KI):
                nc.sync.dma_start(out=xT[:, ic], in_=x_t[:, ic], transpose=True)

        # Two batch halves interleaved at chunk granularity: each consumer
        # chain gets the other half's matmul stream as cover, so ACT/DVE/Pool
        # latency never starves PE.
        p_in_t = {a: {} for a in range(N_HALVES)}

        def emit_in(ha, hc2):
            bs = slice(ha * BH, (ha + 1) * BH)
            pi = ps_in.tile([P, BH], F32, tag="p_in", name=f"p_in_{lbl}_{ha}_{hc2}")
            nch2 = 2 * KH + hc2
            for ic in range(KI):
                nc.tensor.matmul(
                    pi[:], w_ih[:, ic, nch2 * P:(nch2 + 1) * P], xT[:, ic, bs],
                    start=(ic == 0), stop=(ic == KI - 1),
                )
            p_in_t[ha][hc2] = pi

        for _ha in range(N_HALVES):
            emit_in(_ha, 0)

        for hc in range(KH):
            for ha in range(N_HALVES):
                bs = slice(ha * BH, (ha + 1) * BH)
                rc, zc, nch = hc, KH + hc, 2 * KH + hc  # gate chunk ids (of 12)

                def gate_group(gc, tag):
                    pool = ps_r if tag == "r" else ps_z
                    pt = pool.tile([P, BH], F32, tag=tag, name=f"p_{tag}_{lbl}_{ha}_{hc}")
                    for ic in range(KI):
                        nc.tensor.matmul(
                            pt[:], w_ih[:, ic, gc * P:(gc + 1) * P], xT[:, ic, bs],
                            start=(ic == 0), stop=(first and ic == KI - 1),
                        )
                    if not first:
                        for kc in range(KH):
                            nc.tensor.matmul(
                                pt[:], _r(w_hh[:, kc, gc * P:(gc + 1) * P]), _r(h_rd[ha][kc][:]),
                                start=False, stop=(kc == KH - 1),
                            )
                    return pt

                # r group first: its ACT output heads the longest elementwise chain
                p_r = gate_group(rc, "r")
                r_t = gates.tile([P, BH], F32, tag="r")
                nc.scalar.activation(r_t[:], p_r[:], AF.Sigmoid, bias=biases[:, rc:rc + 1])

                p_hn = None
                if not first:
                    p_hn = ps_hn.tile([P, BH], F32, tag="p_hn")
                    for kc in range(KH):
                        nc.tensor.matmul(
                            p_hn[:], _r(w_hh[:, kc, nch * P:(nch + 1) * P]), _r(h_rd[ha][kc][:]),
                            start=(kc == 0), stop=(kc == KH - 1),
                        )
                if hc < KH - 1:
                    emit_in(ha, hc + 1)

                # rh = (p_hn + b_hh_n) * r    (at t=0, h==0 so p_hn == 0)
                rh = gates.tile([P, BH], F32, tag="rh")
                if not first:
                    nc.vector.scalar_tensor_tensor(
                        rh[:], p_hn[:], biases[:, 12 + hc:13 + hc], r_t[:], ALU.add, ALU.mult,
                    )
                else:
                    nc.vector.tensor_scalar_mul(rh[:], r_t[:], biases[:, 12 + hc:13 + hc])

                # n = tanh(rh + p_in + b_ih_n)
                pre = gates.tile([P, BH], F32, tag="pre")
                nc.vector.tensor_add(pre[:], rh[:], p_in_t[ha][hc][:])
                n_t = gates.tile([P, BH], F32, tag="n")
                nc.scalar.activation(n_t[:], pre[:], AF.Tanh, bias=biases[:, 8 + hc:9 + hc])
                d_t = gates.tile([P, BH], F32, tag="d")
                if not first:
                    nc.gpsimd.tensor_sub(d_t[:], h_rd[ha][hc][:], n_t[:])
                else:
                    nc.gpsimd.tensor_scalar_mul(d_t[:], n_t[:], -1.0)

                # z group last: final tail is only z-ACT -> e -> h_add
                p_z = gate_group(zc, "z")
                z_t = gates.tile([P, BH], F32, tag="z")
                nc.scalar.activation(z_t[:], p_z[:], AF.Sigmoid, bias=biases[:, zc:zc + 1])
                # h_new = n + z * (h - n)    (at t=0, h==0 so d = -n)
                e_t = gates.tile([P, BH], F32, tag="e")
                nc.gpsimd.tensor_mul(e_t[:], z_t[:], d_t[:])
                nc.vector.tensor_add(h_wr[ha][hc][:], n_t[:], e_t[:])

    # t=0 statically, t=1..44 in a 2-step hw loop (int8), t=45..48 in a
    # 2-step hw loop (bf16, indexed relative to T_I8), t=49 statically
    step(0, "t0", first=True, h_rd=hbuf[0], h_wr=hbuf[1], i8=True)
    with tc.For_i(1, T_I8, 2) as iv:
        step(iv, "odd", first=False, h_rd=hbuf[1], h_wr=hbuf[0], i8=True)
        step(iv + 1, "even", first=False, h_rd=hbuf[0], h_wr=hbuf[1], i8=True)
    with tc.For_i(0, n_steps - T_I8 - 1, 2) as jv:
        step(jv, "bodd", first=False, h_rd=hbuf[1], h_wr=hbuf[0], i8=False)
        step(jv + 1, "beven", first=False, h_rd=hbuf[0], h_wr=hbuf[1], i8=False)
    step(n_steps - T_I8 - 1, "blast", first=False, h_rd=hbuf[1], h_wr=hbuf[0], i8=False)

    # ---- output projection: out[b, o] = h.T @ w_out.T + b_out ----
    h_fin = hbuf[n_steps % 2]
    o_sb = []
    for oc in range(O_DIM // P):
        ot = gates.tile([P, B_LOCAL], BF16, tag=f"osb{oc}", name=f"osb{oc}")
        for ha in range(N_HALVES):
            p_o = ps_r.tile([P, BH], F32, tag="r", name=f"p_o_{oc}_{ha}")
            for kc in range(KH):
                nc.tensor.matmul(
                    p_o[:], _r(w_out[:, kc, oc * P:(oc + 1) * P]), _r(h_fin[ha][kc][:]),
                    start=(kc == 0), stop=(kc == KH - 1),
                )
            nc.scalar.activation(
                ot[:, ha * BH:(ha + 1) * BH], p_o[:], AF.Identity,
                bias=biases[:, 16 + oc:17 + oc],
            )
        o_sb.append(ot)
    # transpose back to [batch, o] via DMA XBAR and store (bf16)
    outT = gates.tile([P, NB, O_DIM], BF16, tag="outT")
    for oc in range(O_DIM // P):
        nc.sync.dma_start(
            out=outT[:, :, oc * P:(oc + 1) * P], in_=o_sb[oc][:], transpose=True,
        )
    nc.sync.dma_start(out_d.rearrange("(bc p) o -> p bc o", p=P), outT[:])


def build_program(n_steps=T_STEPS):
    nc = bacc.Bacc("TRN2", target_bir_lowering=False, debug=False, num_devices=N_CORES)
    blob_d = nc.dram_tensor("blob", [BLOB_BYTES], mybir.dt.uint8, kind="ExternalInput").ap()
    wsl_d = blob_d.bitcast(F32R)[0:_WS]
    bias_d = blob_d.bitcast(F32)[OFF_BIAS_B // 4:OFF_BIAS_B // 4 + N_BIAS].rearrange(
        "(p j) -> p j", p=P
    )
    wbsl_d = blob_d.bitcast(BF16)[OFF_WBSL_B // 2:OFF_WBSL_B // 2 + _WBS]
    xbf_d = blob_d.bitcast(BF16)[OFF_XBF_B // 2:OFF_XBF_B // 2 + XBF_N].rearrange(
        "(b t i) -> b t i", b=B_LOCAL, t=T_STEPS - T_I8, i=I_DIM
    )
    x8_d = blob_d.bitcast(INT8)[OFF_X8_B:OFF_X8_B + X8_N].rearrange(
        "(b t i) -> b t i", b=B_LOCAL, t=T_I8, i=I_DIM
    )
    wslb_d = nc.dram_tensor("wslb", [W_F32 // N_CORES], F32R, kind="Internal").ap()
    wbslb_d = nc.dram_tensor("wbslb", [W_BF // N_CORES], BF16, kind="Internal").ap()
    wg_d = nc.dram_tensor("wg", [W_F32], F32R, kind="Internal", addr_space="Shared").ap()
    wgb_d = nc.dram_tensor("wgb", [W_BF], BF16, kind="Internal", addr_space="Shared").ap()
    out_d = nc.dram_tensor("out", [B_LOCAL, O_DIM], BF16, kind="ExternalOutput").ap()

    with tile.TileContext(nc) as tc:
        with ExitStack() as ctx:
            _emit(ctx, tc, x8_d, xbf_d, wsl_d, wbsl_d, wslb_d, wbslb_d, wg_d, wgb_d, bias_d, out_d, n_steps)
    nc.compile()
    return nc


def make_host_inputs(w_ih, w_hh, b_ih, b_hh, w_out, b_out, inv_s):
    """Host-side prep: transpose weights into flat AllGather blobs, pack biases.

    w_ih is folded with 1/s so the device consumes x*s (int8 codes or
    pre-scaled bf16 tail) directly: gi = (x*s) @ (w_ih/s).T.
    """
    w_ih_t = np.ascontiguousarray(
        np.asarray(w_ih, dtype=np.float32).T * inv_s
    ).astype(ml_dtypes.bfloat16)
    w_hh_t = np.ascontiguousarray(np.asarray(w_hh, dtype=np.float32).T)
    w_out_t = np.ascontiguousarray(np.asarray(w_out, dtype=np.float32).T)
    b_ih = np.asarray(b_ih, dtype=np.float32)
    b_hh = np.asarray(b_hh, dtype=np.float32)
    b_out = np.asarray(b_out, dtype=np.float32)

    bias_pack = np.zeros((P, 18), dtype=np.float32)
    b_comb = b_ih + b_hh
    for j in range(8):
        bias_pack[:, j] = b_comb[j * P:(j + 1) * P]
    for j in range(4):
        bias_pack[:, 8 + j] = b_ih[2 * H_DIM + j * P:2 * H_DIM + (j + 1) * P]
        bias_pack[:, 12 + j] = b_hh[2 * H_DIM + j * P:2 * H_DIM + (j + 1) * P]
    bias_pack[:, 16] = b_out[:P]
    bias_pack[:, 17] = b_out[P:]
    wpack = np.concatenate([w_hh_t.ravel(), w_out_t.ravel()])
    wpack_bf = w_ih_t.ravel()
    return wpack, wpack_bf, bias_pack


_CACHED_NC = None


def _get_nc():
    global _CACHED_NC
    if _CACHED_NC is None:
        _CACHED_NC = build_program()
    return _CACHED_NC


_BLOB = None
_HF = None


def _fill_blob(x, s, wpack, wpack_bf, bias_pack):
    """Quantize x and pack ALL per-core inputs into one uint8 blob.

    x quantization is minimal-pass for the single-core host: no rint
    (truncation error in the int8 region is invisible through the GRU
    gating) and no clip (the exact absmax guarantees |x*s| <= 127).
    """
    global _BLOB
    if _BLOB is None:
        _BLOB = np.empty((N_CORES, BLOB_BYTES), np.uint8)
    blob = _BLOB
    s = np.float32(s)
    for c in range(N_CORES):
        row = blob[c]
        row[:OFF_BIAS_B].view(np.float32)[:] = wpack[c * _WS:(c + 1) * _WS]
        row[OFF_BIAS_B:OFF_WBSL_B].view(np.float32)[:] = bias_pack.ravel()
        row[OFF_WBSL_B:OFF_XBF_B].view(ml_dtypes.bfloat16)[:] = \
            wpack_bf[c * _WBS:(c + 1) * _WBS]
        xc = x[c * B_LOCAL:(c + 1) * B_LOCAL]
        xbf_c = row[OFF_XBF_B:OFF_X8_B].view(ml_dtypes.bfloat16).reshape(
            B_LOCAL, T_STEPS - T_I8, I_DIM
        )
        np.multiply(xc[:, T_I8:], s, out=xbf_c, casting="unsafe")
        np.multiply(
            xc[:, :T_I8], s,
            out=row[OFF_X8_B:].view(np.int8).reshape(B_LOCAL, T_I8, I_DIM),
            casting="unsafe",
        )
    return blob


LAST_RESULT = None


def kernel(x, w_ih, w_hh, b_ih, b_hh, w_out, b_out, trace=False):
    x = np.asarray(x, dtype=np.float32)
    # One consolidated uint8 blob per core (int8 x codes + scaled bf16 x
    # tail + 1/8 weight slices + biases); weights AllGathered on-device
    absmax = max(float(x.max()), -float(x.min()))
    s = 127.0 / max(absmax, 1e-30)
    wpack, wpack_bf, bias_pack = make_host_inputs(
        w_ih, w_hh, b_ih, b_hh, w_out, b_out, 1.0 / s
    )
    blob = _fill_blob(x, s, wpack, wpack_bf, bias_pack)
    nc = _get_nc()
    in_maps = [{"blob": blob[c]} for c in range(N_CORES)]
    global LAST_RESULT
    LAST_RESULT = run_bass_kernel_spmd(
        nc, in_maps, core_ids=list(range(N_CORES)), trace=trace,
    )
    # gather + bf16->f32 in one pass into a preallocated buffer
    out = np.empty((N_CORES * B_LOCAL, O_DIM), dtype=np.float32)
    for c in range(N_CORES):
        out[c * B_LOCAL:(c + 1) * B_LOCAL] = LAST_RESULT.results[c]["out"]
    return out
